# revision 1
# baseline (speedup 1.0000x reference)
"""Trainium2 Bass kernel for nn_ColorGNN (bipartite GNN message passing).

Math restructuring
------------------
The reference builds a fully-connected bipartite edge list (E = B*C = 262144
edges) and runs a 2-layer MLP per edge. Because the graph is fully connected,
the edge-MLP first layer splits: ef @ W1 = (x_bird @ W1_top)[i] + (x_col @
W1_bot)[j], so the per-edge hidden is relu(A[i] + Bm[j] + b1) and the
scatter-add pushes through W2:
    aggr_bird[i]  = (sum_j relu(A[i]+Bm[j]+b1)) @ W2 + C*b2
    aggr_color[j] = (sum_i relu(A[i]+Bm[j]+b1)) @ W2 + B*b2
No E-sized tensor is ever materialized.

top_k with k=B=C selects every element, so filtered == probs and mask == 1;
the sort only permutes the final output.  The device computes combined =
(x_bird @ cpw + cpb) * probs; the host does the argsort/gather.

Sharding (8 cores)
------------------
Colors are sharded 64 per core. Each core keeps the full bird features
(replicated) and only its 64 colors' features. S_color is then complete
locally (sum over ALL birds); only S_bird needs one 256 KB AllReduce per
layer. All per-core differences enter through host-sliced input tensors, so
the program is pure SPMD.

Per-core layout: hidden dim H=128 lives on SBUF partitions; node features are
kept transposed (H x nodes). Inner loop per own-color t:
  T_t = relu(A + Bb[:, t]) in bf16   (ScalarE activation w/ per-partition
                                      bias for the first N_AF tiles --
                                      fused free-dim accum gives S_color --
                                      VectorE tensor_scalar for the rest,
                                      followed by a VectorE tensor_reduce)
  PSUM += W2_hi^T @ T_t and W2_lo^T @ T_t  (TensorE; W2 is split into two
            bf16 halves so the weight rounding error is ~2^-16; four PSUM
            banks rotate so back-to-back matmuls pipeline at ~210ns)
This folds the S_bird scatter-add AND the edge-MLP second matmul into the
same PE accumulation; S_bird itself is never materialized.

HW-debugging notes (measured on trn2 via axon):
  - scalar_tensor_tensor with accum_out crashes the device (unrecoverable);
    activation with accum_out works.
  - fp32 matmuls run at 1/4 rate; bf16 rhs streams 1 col/cycle.
  - back-to-back matmuls accumulating into the SAME PSUM bank serialize
    (~510ns); alternating banks restores ~210ns.
  - --enable-ldw-opt=false in this env: every matmul pays its LDWEIGHTS.
"""

import numpy as np
from ml_dtypes import bfloat16 as _bf16

import concourse.bass as bass
import concourse.bacc as bacc
import concourse.mybir as mybir
import concourse.tile as tile
from concourse import bass_utils

B = 512  # birds
C = 512  # colors
H = 128  # hidden
L = 3  # layers
NCORES = 8
CPC = C // NCORES  # colors per core

F32 = mybir.dt.float32
AF = mybir.ActivationFunctionType
ALU = mybir.AluOpType

BF16 = mybir.dt.bfloat16

# t-indices handled by ScalarE (rest go to VectorE). Contiguous halves so the
# two engines write disjoint S_color tiles.
# Inner-loop tile roles (HW-measured costs):
#   af: ScalarE relu+fused-accum        ~1172ns ACT
#   ap: ScalarE relu ~518ns ACT + VectorE reduce ~594ns DVE
#   gp: GpSimd tensor_scalar ~? POOL   + VectorE reduce ~594ns DVE
#   dd: VectorE tensor_scalar ~716ns + reduce ~594ns, all DVE
# Roles are assigned in contiguous blocks [af | ap | gp | dd].
N_AF = 22
N_AP = 0
N_GP = 0


def _build(repeat: int = 1, variant: str = "full", num_devices: int = NCORES):
    # variant: "full" the real kernel; "v0" init+final only (debug);
    # "nocc" AllReduces replaced by local DRAM copies and the repeat loop
    # runs under a hardware For_i (used only for timing).
    nc = bacc.Bacc(
        "TRN2", target_bir_lowering=False, debug=False, num_devices=num_devices
    )

    # ---- I/O ----
    inp = {}

    def di(name, shape):
        inp[name] = nc.dram_tensor(name, list(shape), F32, kind="ExternalInput")
        return inp[name]

    di("probsT", (CPC, B))  # probs.T slice: my colors x all birds (final stage)
    di("probsTF", (C, B))  # full probs.T (initial projection)
    di("npwF", (C, H))  # full node_proj_w
    di("xc0", (H, CPC))  # initial color features (transposed)
    di("npb", (H, 1))  # node_proj_b
    for l in range(L):
        di(f"e1t{l}", (H, H))  # edge_w1[l][:H]
        di(f"e1b{l}", (H, H))  # edge_w1[l][H:]
        di(f"eb1_{l}", (H, 1))
        di(f"e2_{l}", (H, H))
        di(f"e2lo{l}", (H, H))  # e2 - bf16(e2), for the hi/lo split
        di(f"eb2f{l}", (H, 1))  # 512*edge_b2[l]
        di(f"eb2p{l}", (H, 1))  # 512*edge_b2[l] / 8
        di(f"n1x{l}", (H, H))  # node_w1[l][:H]
        di(f"n1a{l}", (H, H))  # node_w1[l][H:]
        di(f"nb1_{l}", (H, 1))
        di(f"n2_{l}", (H, H))
        di(f"nb2_{l}", (H, 1))
    di("cpw", (H, CPC))  # color_proj_w columns for my colors
    di("cpb", (CPC, 1))
    out_dram = nc.dram_tensor("out", [CPC, B], F32, kind="ExternalOutput")

    rg = [list(range(NCORES))]

    def dma4(dst, src):
        # split a (P, 512) transfer into 4 column chunks -> 4 DMA queues
        n = src.shape[-1]
        step = n // 4
        for k in range(4):
            nc.sync.dma_start(
                dst[:, k * step : (k + 1) * step],
                src[:, k * step : (k + 1) * step],
            )

    with tile.TileContext(nc) as tc:
        with (
            tc.tile_pool(name="const", bufs=1) as cpool,
            tc.tile_pool(name="work", bufs=2) as wpool,
            tc.tile_pool(name="tbuf", bufs=17) as tpool,
            tc.tile_pool(name="psA", bufs=2, space="PSUM") as psA,
            tc.tile_pool(name="psS", bufs=1, space="PSUM") as psS,
            tc.tile_pool(name="ps64", bufs=2, space="PSUM") as ps64,
            tc.tile_pool(name="dram", bufs=1, space="DRAM") as dpool,
        ):
            # ---- load constants to SBUF ----
            sb = {}
            # the initial projection gates everything: load its operands
            # first, chunk-interleaved so matmul k unblocks progressively
            for k in range(4):
                for name in ("npwF", "probsTF"):
                    t = inp[name]
                    sl = cpool.tile(
                        [128, t.shape[1]], F32, tag=f"{name}{k}",
                        name=f"{name}{k}",
                    )
                    nc.sync.dma_start(sl[:], t[128 * k : 128 * (k + 1), :])
                    sb[f"{name}{k}"] = sl
            for name, t in inp.items():
                if name in ("probsTF", "npwF"):
                    continue
                s = cpool.tile(list(t.shape), F32, tag=name)
                nc.sync.dma_start(s[:], t[:])
                sb[name] = s

            # bf16 hi/lo split of the edge W2 weights (stationary operands
            # of the inner-loop accumulation; rhs T tiles are bf16). Two
            # streams keep the weight error at ~2^-16.
            for l in range(L):
                e2hi = cpool.tile([H, H], BF16, tag=f"e2hi{l}")
                nc.vector.tensor_copy(e2hi[:], sb[f"e2_{l}"][:])
                sb[f"e2hi{l}"] = e2hi
                e2lo = cpool.tile([H, H], BF16, tag=f"e2lobf{l}")
                nc.vector.tensor_copy(e2lo[:], sb[f"e2lo{l}"][:])
                sb[f"e2lobf{l}"] = e2lo

            def emit_rep():
                # ---- init: x_bird^T = npw^T @ probs^T + npb, computed fully
                # locally (contraction over all colors in 4 chunks)
                ps = psA.tile([H, B], F32, tag="mm512")
                for k in range(4):
                    nc.tensor.matmul(
                        ps[:], sb[f"npwF{k}"][:], sb[f"probsTF{k}"][:],
                        start=(k == 0), stop=(k == 3),
                    )
                xbT = wpool.tile([H, B], F32, tag="xbT")
                nc.scalar.activation(
                    xbT[:], ps[:], AF.Identity, bias=sb["npb"][:]
                )
                xcT = sb["xc0"]

                for l in range(L if variant != "v0" else 0):
                    # A = relu-input contribution of birds: (H, B),
                    # 128-col slices pipelined behind the xbT slices
                    psa = psA.tile([H, B], F32, tag="mm512")
                    A_sb = wpool.tile([H, B], F32, tag="A_sb")
                    for k in range(4):
                        csl = slice(128 * k, 128 * (k + 1))
                        nc.tensor.matmul(
                            psa[:, csl], sb[f"e1t{l}"][:], xbT[:, csl]
                        )
                        nc.scalar.copy(A_sb[:, csl], psa[:, csl])
                    # Bb = W1_bot^T @ xc_mine + eb1 : (H, CPC)
                    psb = ps64.tile([H, CPC], F32, tag="mm64")
                    nc.tensor.matmul(psb[:], sb[f"e1b{l}"][:], xcT[:])
                    Bb = wpool.tile([H, CPC], F32, tag="Bb")
                    nc.scalar.activation(
                        Bb[:], psb[:], AF.Identity, bias=sb[f"eb1_{l}"][:]
                    )

                    # bird-MLP x-contribution: only needs xbT, emit early
                    # so the post-AllReduce tail is short
                    ph1b = psA.tile([H, B], F32, tag="mm512")
                    nc.tensor.matmul(
                        ph1b[:], sb[f"n1x{l}"][:], xbT[:], start=True, stop=False
                    )

                    # ---- inner loop over my colors ----
                    # PE folds the S_bird scatter-add and the W2 matmul into
                    # one accumulation: psum += W2^T @ T_t, alternating two
                    # banks so consecutive matmuls pipeline.
                    ps_Sb = [psS.tile([H, B], F32, tag=f"S{k}", name=f"S{k}") for k in range(4)]
                    scol_a = wpool.tile([H, max(N_AF, 1)], F32, tag="scol_a")
                    scol_d = wpool.tile([H, CPC - N_AF], F32, tag="scol_d")
                    # Tiles are produced in groups of G; per group PE runs
                    # G hi-matmuls then G lo-matmuls so the stationary weight
                    # reloads only twice per group (LDWEIGHTS dedup).
                    G = 8
                    # Interleave ACT-role and DVE-role tiles in emission order
                    # (colors keep their contiguous role blocks for S_color):
                    # PE consumes T tiles in emission order, so block order
                    # would throttle the pool to one producer's rate per phase.
                    order = sorted(
                        range(CPC),
                        key=lambda t: (t + 0.5) / N_AF if t < N_AF
                        else (t - N_AF + 0.5) / (CPC - N_AF),
                    )
                    for g in range(0, CPC, G):
                        Ts = []
                        for sI in range(g, g + G):
                            t = order[sI]
                            T_t = tpool.tile([H, B], BF16, tag="T", name=f"T{t}")
                            bias = Bb[:, t : t + 1]
                            if t < N_AF:
                                nc.scalar.activation(
                                    T_t[:], A_sb[:], AF.Relu, bias=bias,
                                    accum_out=scol_a[:, t : t + 1],
                                )
                            else:
                                if t < N_AF + N_AP:
                                    nc.scalar.activation(
                                        T_t[:], A_sb[:], AF.Relu, bias=bias
                                    )
                                elif t < N_AF + N_AP + N_GP:
                                    nc.gpsimd.tensor_scalar(
                                        out=T_t[:], in0=A_sb[:], scalar1=bias,
                                        scalar2=0.0, op0=ALU.add, op1=ALU.max,
                                    )
                                else:
                                    # stt-with-accum_out crashes HW; plain
                                    # tensor_scalar + reduce instead
                                    nc.vector.tensor_scalar(
                                        out=T_t[:], in0=A_sb[:], scalar1=bias,
                                        scalar2=0.0, op0=ALU.add, op1=ALU.max,
                                    )
                                nc.vector.tensor_reduce(
                                    out=scol_d[:, t - N_AF : t - N_AF + 1],
                                    in_=T_t[:], axis=mybir.AxisListType.X,
                                    op=ALU.add,
                                )
                            Ts.append(T_t)
                        for k, sI in enumerate(range(g, g + G)):
                            nc.tensor.matmul(
                                ps_Sb[sI % 2][:], sb[f"e2hi{l}"][:], Ts[k][:],
                                start=(sI < 2), stop=(sI >= CPC - 2),
                            )
                        for k, sI in enumerate(range(g, g + G)):
                            nc.tensor.matmul(
                                ps_Sb[2 + sI % 2][:], sb[f"e2lobf{l}"][:], Ts[k][:],
                                start=(sI < 2), stop=(sI >= CPC - 2),
                            )

                    # ---- color side (fully local) ----
                    psac = ps64.tile([H, CPC], F32, tag="mm64")
                    if N_AF > 0:
                        nc.tensor.matmul(
                            psac[:, :N_AF], sb[f"e2_{l}"][:], scol_a[:, :N_AF]
                        )
                    nc.tensor.matmul(
                        psac[:, N_AF:], sb[f"e2_{l}"][:], scol_d[:]
                    )
                    aggrC = wpool.tile([H, CPC], F32, tag="aggrC")
                    nc.scalar.activation(
                        aggrC[:], psac[:], AF.Identity, bias=sb[f"eb2f{l}"][:]
                    )
                    # color node MLP
                    ph1c = ps64.tile([H, CPC], F32, tag="mm64")
                    nc.tensor.matmul(
                        ph1c[:], sb[f"n1x{l}"][:], xcT[:], start=True, stop=False
                    )
                    nc.tensor.matmul(
                        ph1c[:], sb[f"n1a{l}"][:], aggrC[:], start=False, stop=True
                    )
                    h1c = wpool.tile([H, CPC], F32, tag="h1c")
                    nc.scalar.activation(
                        h1c[:], ph1c[:], AF.Relu, bias=sb[f"nb1_{l}"][:]
                    )
                    px2c = ps64.tile([H, CPC], F32, tag="mm64")
                    nc.tensor.matmul(px2c[:], sb[f"n2_{l}"][:], h1c[:])
                    xcT_new = wpool.tile([H, CPC], F32, tag="xcT")
                    nc.scalar.activation(
                        xcT_new[:], px2c[:], AF.Identity, bias=sb[f"nb2_{l}"][:]
                    )

                    # ---- bird side: combine the two PSUM banks, + bias,
                    # then AllReduce ----
                    aggrB_part = wpool.tile([H, B], F32, tag="aggrB_part")
                    nc.scalar.activation(
                        aggrB_part[:], ps_Sb[0][:], AF.Identity,
                        bias=sb[f"eb2p{l}"][:],
                    )
                    for k in range(1, 4):
                        nc.vector.tensor_tensor(
                            aggrB_part[:], aggrB_part[:], ps_Sb[k][:], ALU.add
                        )
                    ci = dpool.tile([H, B], F32, tag=f"cc{l}_in")
                    co = dpool.tile([H, B], F32, tag=f"cc{l}_out")
                    dma4(ci, aggrB_part)
                    if variant == "nocc":
                        dma4(co, ci)
                    else:
                        nc.gpsimd.collective_compute(
                            "AllReduce", ALU.add, replica_groups=rg,
                            ins=[ci[:]], outs=[co[:]],
                        )
                    aggrB = wpool.tile([H, B], F32, tag="aggrB")
                    dma4(aggrB, co)
                    # bird node MLP (replicated)
                    h1b = wpool.tile([H, B], F32, tag="h1b")
                    px2b = psA.tile([H, B], F32, tag="mm512")
                    xbT_new = wpool.tile([H, B], F32, tag="xbT")
                    for k in range(4):
                        csl = slice(128 * k, 128 * (k + 1))
                        nc.tensor.matmul(
                            ph1b[:, csl], sb[f"n1a{l}"][:], aggrB[:, csl],
                            start=False, stop=True,
                        )
                        nc.scalar.activation(
                            h1b[:, csl], ph1b[:, csl], AF.Relu,
                            bias=sb[f"nb1_{l}"][:],
                        )
                        nc.tensor.matmul(
                            px2b[:, csl], sb[f"n2_{l}"][:], h1b[:, csl]
                        )
                        nc.scalar.activation(
                            xbT_new[:, csl], px2b[:, csl], AF.Identity,
                            bias=sb[f"nb2_{l}"][:],
                        )
                    xbT = xbT_new
                    xcT = xcT_new

                # ---- final: combined^T rows for my colors ----
                pssc = psA.tile([CPC, B], F32, tag="mm512")
                nc.tensor.matmul(pssc[:], sb["cpw"][:], xbT[:])
                out_sb = wpool.tile([CPC, B], F32, tag="out_sb")
                nc.vector.scalar_tensor_tensor(
                    out=out_sb[:], in0=pssc[:], scalar=sb["cpb"][:],
                    in1=sb["probsT"][:], op0=ALU.add, op1=ALU.mult,
                )
                dma4(out_dram, out_sb)

            if variant == "nocc" and repeat > 1:
                with tc.For_i(0, repeat, 1) as _i:
                    emit_rep()
            else:
                for _rep in range(repeat):
                    emit_rep()

    nc.compile()
    return nc


_BUILT = {}


def _get_built(repeat: int = 1):
    if repeat not in _BUILT:
        _BUILT[repeat] = _build(repeat)
    return _BUILT[repeat]


def make_in_maps(probs, node_proj_w, node_proj_b, edge_w1, edge_b1, edge_w2,
                 edge_b2, node_w1, node_b1, node_w2, node_b2, color_proj_w,
                 color_proj_b):
    f = lambda x: np.ascontiguousarray(np.asarray(x, dtype=np.float32))
    probs = f(probs)
    probsT = probs.T
    in_maps = []
    for c in range(NCORES):
        sl = slice(CPC * c, CPC * (c + 1))
        m = {
            "probsT": f(probsT[sl]),
            "probsTF": f(probsT),
            "npwF": f(node_proj_w),
            "xc0": f((np.asarray(node_proj_w)[sl] + np.asarray(node_proj_b)).T),
            "npb": f(node_proj_b).reshape(H, 1),
            "cpw": f(np.asarray(color_proj_w)[:, sl]),
            "cpb": f(np.asarray(color_proj_b)[sl]).reshape(CPC, 1),
        }
        for l in range(L):
            m[f"e1t{l}"] = f(edge_w1[l][:H])
            m[f"e1b{l}"] = f(edge_w1[l][H:])
            m[f"eb1_{l}"] = f(edge_b1[l]).reshape(H, 1)
            e2 = f(edge_w2[l])
            m[f"e2_{l}"] = e2
            m[f"e2lo{l}"] = (e2 - e2.astype(_bf16).astype(np.float32))
            m[f"eb2f{l}"] = f(512.0 * np.asarray(edge_b2[l])).reshape(H, 1)
            m[f"eb2p{l}"] = f(512.0 * np.asarray(edge_b2[l]) / 8.0).reshape(H, 1)
            m[f"n1x{l}"] = f(node_w1[l][:H])
            m[f"n1a{l}"] = f(node_w1[l][H:])
            m[f"nb1_{l}"] = f(node_b1[l]).reshape(H, 1)
            m[f"n2_{l}"] = f(node_w2[l])
            m[f"nb2_{l}"] = f(node_b2[l]).reshape(H, 1)
        in_maps.append(m)
    return in_maps


def finish(probs, core_outs):
    combinedT = np.concatenate(core_outs, axis=0)  # (C, B)
    combined = combinedT.T  # (B, C)
    probs = np.asarray(probs, dtype=np.float32)
    idx = np.argsort(-probs, axis=1, kind="stable")
    cost = 1.0 - np.take_along_axis(combined, idx, axis=1)
    return cost.astype(np.float32)


def kernel(probs, node_proj_w, node_proj_b, edge_w1, edge_b1, edge_w2,
           edge_b2, node_w1, node_b1, node_w2, node_b2, color_proj_w,
           color_proj_b):
    nc = _get_built()
    in_maps = make_in_maps(
        probs, node_proj_w, node_proj_b, edge_w1, edge_b1, edge_w2, edge_b2,
        node_w1, node_b1, node_w2, node_b2, color_proj_w, color_proj_b,
    )
    res = bass_utils.run_bass_kernel_spmd(nc, in_maps, list(range(NCORES)))
    return finish(probs, [r["out"] for r in res.results])



# revision 9
# speedup vs baseline: 1.6048x; 1.6048x over previous
"""Trainium2 Bass kernel for nn_ColorGNN (bipartite GNN message passing).

Math restructuring
------------------
The reference builds a fully-connected bipartite edge list (E = B*C = 262144
edges) and runs a 2-layer MLP per edge. Because the graph is fully connected,
the edge-MLP first layer splits: ef @ W1 = (x_bird @ W1_top)[i] + (x_col @
W1_bot)[j], so the per-edge hidden is relu(A[i] + Bm[j] + b1) and the
scatter-add pushes through W2:
    aggr_bird[i]  = (sum_j relu(A[i]+Bm[j]+b1)) @ W2 + C*b2
    aggr_color[j] = (sum_i relu(A[i]+Bm[j]+b1)) @ W2 + B*b2
No E-sized tensor is ever materialized.

top_k with k=B=C selects every element, so filtered == probs and mask == 1;
the sort only permutes the final output.  The device computes combined =
(x_bird @ cpw + cpb) * probs; the host does the argsort/gather.

Sharding (8 cores)
------------------
Colors are sharded 64 per core. Each core keeps the full bird features
(replicated) and only its 64 colors' features. S_color is then complete
locally (sum over ALL birds); only S_bird needs one 256 KB AllReduce per
layer. All per-core differences enter through host-sliced input tensors, so
the program is pure SPMD.

Per-core layout: hidden dim H=128 lives on SBUF partitions; node features are
kept transposed (H x nodes). Inner loop per own-color t:
  T_t = relu(A + Bb[:, t]) in bf16   (ScalarE activation w/ per-partition
                                      bias for the first N_AF tiles --
                                      fused free-dim accum gives S_color --
                                      VectorE tensor_scalar for the rest,
                                      followed by a VectorE tensor_reduce)
  PSUM += W2_hi^T @ T_t and W2_lo^T @ T_t  (TensorE; W2 is split into two
            bf16 halves so the weight rounding error is ~2^-16; four PSUM
            banks rotate so back-to-back matmuls pipeline at ~210ns)
This folds the S_bird scatter-add AND the edge-MLP second matmul into the
same PE accumulation; S_bird itself is never materialized.

HW-debugging notes (measured on trn2 via axon):
  - scalar_tensor_tensor with accum_out crashes the device (unrecoverable);
    activation with accum_out works.
  - fp32 matmuls run at 1/4 rate; bf16 rhs streams 1 col/cycle.
  - back-to-back matmuls accumulating into the SAME PSUM bank serialize
    (~510ns); alternating banks restores ~210ns.
  - --enable-ldw-opt=false in this env: every matmul pays its LDWEIGHTS.
"""

import numpy as np
from ml_dtypes import bfloat16 as _bf16

import concourse.bass as bass
import concourse.bacc as bacc
import concourse.mybir as mybir
import concourse.tile as tile
from concourse import bass_utils

# ---- custom DVE op: out = relu(in0 + s0); accum_out = s1 + sum(out) ----
# Fuses the per-color produce pass and the S_color free-dim reduce into one
# 1x DVE instruction (~0.6us/tile vs ~1.3us for tensor_scalar+tensor_reduce).
from operator import add as _opadd
from concourse import dve_ops as _dve_ops
from concourse.dve_spec import (
    Spec as _Spec, Src0 as _Src0, C0 as _C0, C1 as _C1,
    relu as _dve_relu, _has_src1, lower as _dve_lower,
)
from concourse.dve_uop import DveOpSpec as _DveOpSpec
from concourse.dve_ops import DveOp as _DveOp


def _rbr_ref(in0, in1, s0, s1, imm2):
    b = np.maximum(in0.astype(np.float32) + s0, 0).astype(np.float32)
    return b, s1 + b.reshape(b.shape[0], -1).sum(axis=-1, keepdims=True)


def _register_rbr():
    name = "RELU_BIAS_REDUCE_ANT"
    if name in _dve_ops._SUB_OPCODE_FOR_NAME:
        return next(o for o in _dve_ops.OPS if o.name == name)
    spec = _Spec(
        body=_dve_relu(_Src0 + _C0), accum=_opadd, accum_init=_C1,
        reference=_rbr_ref,
    )
    row = _dve_ops._CUSTOM_DVE_ROW_BASE + len(_dve_ops.OPS)
    shas = {
        ver: _DveOpSpec(
            name=name, opcode=row, uops=_dve_lower(spec, ver=ver),
            rd1_en=_has_src1(spec),
        ).sha(ver)
        for ver in ("v3", "v4")
    }
    op = _DveOp(name, spec, False, shas)
    _dve_ops.OPS.append(op)
    _dve_ops._SUB_OPCODE_FOR_NAME[name] = row
    _dve_ops.CUSTOM_DVE_SPECS[name] = spec
    return op


RBR = _register_rbr()

B = 512  # birds
C = 512  # colors
H = 128  # hidden
L = 3  # layers
NCORES = 8
CPC = C // NCORES  # colors per core

F32 = mybir.dt.float32
AF = mybir.ActivationFunctionType
ALU = mybir.AluOpType

BF16 = mybir.dt.bfloat16

# t-indices handled by ScalarE (rest go to VectorE). Contiguous halves so the
# two engines write disjoint S_color tiles.
# Inner-loop tile roles (HW-measured costs):
#   af: ScalarE relu+fused-accum        ~1172ns ACT
#   dd: VectorE custom fused relu+accum (USE_CUST) or tensor_scalar+reduce
# Roles are assigned in contiguous blocks [af | dd].
N_AF = 21       # ScalarE share, layers 0-1 (accum fused, 727ns/tile + misc)
N_AF_LAST = 25  # ScalarE share, last layer (no accum needed, 540ns/tile)
N_AP = 0
N_GP = 0
USE_CUST = True   # custom DVE op for dd-role tiles
DROP_LO = True    # single bf16 stream for the inner W2 matmul


def _build(repeat: int = 1, variant: str = "full", num_devices: int = NCORES):
    # variant: "full" the real kernel; "v0" init+final only (debug);
    # "nocc" AllReduces replaced by local DRAM copies and the repeat loop
    # runs under a hardware For_i (used only for timing).
    nc = bacc.Bacc(
        "TRN2", target_bir_lowering=False, debug=False, num_devices=num_devices
    )

    # ---- I/O ----
    inp = {}

    def di(name, shape):
        inp[name] = nc.dram_tensor(name, list(shape), F32, kind="ExternalInput")
        return inp[name]

    def di16(name, shape):
        inp[name] = nc.dram_tensor(name, list(shape), BF16, kind="ExternalInput")
        return inp[name]

    di("probsT", (CPC, B))  # probs.T slice: my colors x all birds (final stage)
    di16("probsTF", (C, B))  # full probs.T (initial projection), bf16
    di16("npwF", (C, H))  # full node_proj_w, bf16
    di("xc0", (H, CPC))  # initial color features (transposed)
    di("npb", (H, 1))  # node_proj_b
    for l in range(L):
        di16(f"e1t16{l}", (H, H))  # edge_w1[l][:H], bf16 (bird-side lhsT)
        di(f"e1b{l}", (H, H))  # edge_w1[l][H:]
        di(f"eb1_{l}", (H, 1))
        di(f"e2_{l}", (H, H))
        di(f"eb2f{l}", (H, 1))  # 512*edge_b2[l]
        di(f"eb2p{l}", (H, 1))  # 512*edge_b2[l] / 8
        di16(f"n1x16{l}", (H, H))  # node_w1[l][:H], bf16 (bird-side lhsT)
        di(f"n1x{l}", (H, H))  # node_w1[l][:H] fp32 (color side)
        di(f"n1a{l}", (H, H))  # node_w1[l][H:]
        di(f"nb1_{l}", (H, 1))
        di16(f"n2_16{l}", (H, H))  # node_w2[l] bf16 (bird side)
        di(f"n2_{l}", (H, H))  # fp32 (color side)
        di(f"nb2_{l}", (H, 1))
    di16("cpw16", (H, CPC))  # color_proj_w columns for my colors, bf16
    di("cpb", (CPC, 1))
    out_dram = nc.dram_tensor("out", [CPC, B], F32, kind="ExternalOutput")

    rg = [list(range(NCORES))]

    def dma4(dst, src):
        # split a (P, 512) transfer into 4 column chunks -> 4 DMA queues
        n = src.shape[-1]
        step = n // 4
        for k in range(4):
            nc.sync.dma_start(
                dst[:, k * step : (k + 1) * step],
                src[:, k * step : (k + 1) * step],
            )

    with tile.TileContext(nc) as tc:
        with (
            tc.tile_pool(name="const", bufs=1) as cpool,
            tc.tile_pool(name="work", bufs=2) as wpool,
            tc.tile_pool(name="tbuf", bufs=17) as tpool,
            tc.tile_pool(name="psA", bufs=2, space="PSUM") as psA,
            tc.tile_pool(name="psS", bufs=1, space="PSUM") as psS,
            tc.tile_pool(name="ps64", bufs=2, space="PSUM") as ps64,
            tc.tile_pool(name="dram", bufs=1, space="DRAM") as dpool,
        ):
            # ---- load constants to SBUF ----
            sb = {}
            # the initial projection gates everything: load its operands
            # first, chunk-interleaved so matmul k unblocks progressively
            for k in range(4):
                for name in ("npwF", "probsTF"):
                    t = inp[name]
                    sl = cpool.tile(
                        [128, t.shape[1]], BF16, tag=f"{name}{k}",
                        name=f"{name}{k}",
                    )
                    nc.sync.dma_start(sl[:], t[128 * k : 128 * (k + 1), :])
                    sb[f"{name}{k}"] = sl
            for name, t in inp.items():
                if name in ("probsTF", "npwF"):
                    continue
                s = cpool.tile(list(t.shape), t.dtype, tag=name)
                nc.sync.dma_start(s[:], t[:])
                sb[name] = s

            # bf16 (hi) stream of the edge W2 weights (stationary operands
            # of the inner-loop accumulation; rhs T tiles are bf16). With
            # DROP_LO the 2e-2 tolerance absorbs the ~2^-9 weight rounding.
            for l in range(L):
                e2hi = cpool.tile([H, H], BF16, tag=f"e2hi{l}")
                nc.vector.tensor_copy(e2hi[:], sb[f"e2_{l}"][:])
                sb[f"e2hi{l}"] = e2hi
                if not DROP_LO:
                    e2lo = cpool.tile([H, H], BF16, tag=f"e2lobf{l}")
                    nc.vector.tensor_copy(e2lo[:], sb[f"e2lo{l}"][:])
                    sb[f"e2lobf{l}"] = e2lo

            def emit_rep():
                # ---- init: x_bird^T = npw^T @ probs^T + npb, computed fully
                # locally (contraction over all colors in 4 bf16 chunks)
                ps = psA.tile([H, B], F32, tag="mm512")
                for k in range(4):
                    nc.tensor.matmul(
                        ps[:], sb[f"npwF{k}"][:], sb[f"probsTF{k}"][:],
                        start=(k == 0), stop=(k == 3),
                    )
                xbT = wpool.tile([H, B], BF16, tag="xbT")
                nc.scalar.activation(
                    xbT[:], ps[:], AF.Identity, bias=sb["npb"][:]
                )
                xcT = sb["xc0"]

                for l in range(L if variant != "v0" else 0):
                    last = l == L - 1
                    n_af = N_AF_LAST if last else N_AF
                    # A = relu-input contribution of birds: (H, B) bf16
                    psa = psA.tile([H, B], F32, tag="mm512")
                    A_sb = wpool.tile([H, B], BF16, tag="A_sb")
                    for k in range(4):
                        csl = slice(128 * k, 128 * (k + 1))
                        nc.tensor.matmul(
                            psa[:, csl], sb[f"e1t16{l}"][:], xbT[:, csl]
                        )
                        nc.scalar.copy(A_sb[:, csl], psa[:, csl])
                    # Bb = W1_bot^T @ xc_mine + eb1 : (H, CPC)
                    psb = ps64.tile([H, CPC], F32, tag="mm64")
                    nc.tensor.matmul(psb[:], sb[f"e1b{l}"][:], xcT[:])
                    Bb = wpool.tile([H, CPC], F32, tag="Bb")
                    nc.scalar.activation(
                        Bb[:], psb[:], AF.Identity, bias=sb[f"eb1_{l}"][:]
                    )

                    # bird-MLP x-contribution: only needs xbT, emit early
                    # so the post-AllReduce tail is short
                    ph1b = psA.tile([H, B], F32, tag="mm512")
                    nc.tensor.matmul(
                        ph1b[:], sb[f"n1x16{l}"][:], xbT[:], start=True,
                        stop=False,
                    )

                    # ---- inner loop over my colors ----
                    # PE folds the S_bird scatter-add and the W2 matmul into
                    # one accumulation: psum += W2^T @ T_t, alternating two
                    # banks so consecutive matmuls pipeline. On the last
                    # layer S_color (and the whole color side) is dead code:
                    # produce-only tiles, no accumulators.
                    ps_Sb = [psS.tile([H, B], F32, tag=f"S{k}", name=f"S{k}") for k in range(2)]
                    if not last:
                        scol_a = wpool.tile(
                            [H, max(n_af, 1)], F32, tag="scol_a"
                        )
                        scol_d = wpool.tile(
                            [H, CPC - n_af], F32, tag="scol_d"
                        )
                    G = 8
                    # Interleave ACT-role and DVE-role tiles in emission order
                    # (colors keep their contiguous role blocks for S_color):
                    # PE consumes T tiles in emission order, so block order
                    # would throttle the pool to one producer's rate per phase.
                    order = sorted(
                        range(CPC),
                        key=lambda t: (t + 0.5) / n_af if t < n_af
                        else (t - n_af + 0.5) / (CPC - n_af),
                    )
                    for g in range(0, CPC, G):
                        Ts = []
                        for sI in range(g, g + G):
                            t = order[sI]
                            T_t = tpool.tile([H, B], BF16, tag="T", name=f"T{t}")
                            bias = Bb[:, t : t + 1]
                            if t < n_af:
                                if last:
                                    nc.scalar.activation(
                                        T_t[:], A_sb[:], AF.Relu, bias=bias
                                    )
                                else:
                                    nc.scalar.activation(
                                        T_t[:], A_sb[:], AF.Relu, bias=bias,
                                        accum_out=scol_a[:, t : t + 1],
                                    )
                            elif USE_CUST:
                                nc.vector._custom_dve(
                                    RBR, out=T_t[:], in0=A_sb[:], s0=bias,
                                    s1=0.0,
                                    accum_out=None if last
                                    else scol_d[:, t - n_af : t - n_af + 1],
                                )
                            else:
                                nc.vector.tensor_scalar(
                                    out=T_t[:], in0=A_sb[:], scalar1=bias,
                                    scalar2=0.0, op0=ALU.add, op1=ALU.max,
                                )
                                if not last:
                                    nc.vector.tensor_reduce(
                                        out=scol_d[:, t - n_af : t - n_af + 1],
                                        in_=T_t[:], axis=mybir.AxisListType.X,
                                        op=ALU.add,
                                    )
                            Ts.append(T_t)
                        for k, sI in enumerate(range(g, g + G)):
                            nc.tensor.matmul(
                                ps_Sb[sI % 2][:], sb[f"e2hi{l}"][:], Ts[k][:],
                                start=(sI < 2), stop=(sI >= CPC - 2),
                            )

                    # ---- color side (fully local; dead on last layer) ----
                    if not last:
                        psac = ps64.tile([H, CPC], F32, tag="mm64")
                        if n_af > 0:
                            nc.tensor.matmul(
                                psac[:, :n_af], sb[f"e2_{l}"][:],
                                scol_a[:, :n_af],
                            )
                        nc.tensor.matmul(
                            psac[:, n_af:], sb[f"e2_{l}"][:], scol_d[:]
                        )
                        aggrC = wpool.tile([H, CPC], F32, tag="aggrC")
                        nc.scalar.activation(
                            aggrC[:], psac[:], AF.Identity,
                            bias=sb[f"eb2f{l}"][:],
                        )
                        # color node MLP
                        ph1c = ps64.tile([H, CPC], F32, tag="mm64")
                        nc.tensor.matmul(
                            ph1c[:], sb[f"n1x{l}"][:], xcT[:], start=True,
                            stop=False,
                        )
                        nc.tensor.matmul(
                            ph1c[:], sb[f"n1a{l}"][:], aggrC[:], start=False,
                            stop=True,
                        )
                        h1c = wpool.tile([H, CPC], F32, tag="h1c")
                        nc.scalar.activation(
                            h1c[:], ph1c[:], AF.Relu, bias=sb[f"nb1_{l}"][:]
                        )
                        px2c = ps64.tile([H, CPC], F32, tag="mm64")
                        nc.tensor.matmul(px2c[:], sb[f"n2_{l}"][:], h1c[:])
                        xcT_new = wpool.tile([H, CPC], F32, tag="xcT")
                        nc.scalar.activation(
                            xcT_new[:], px2c[:], AF.Identity,
                            bias=sb[f"nb2_{l}"][:],
                        )

                    # ---- bird side: combine the two PSUM banks, + bias,
                    # then AllReduce ----
                    aggrB_part = wpool.tile([H, B], F32, tag="aggrB_part")
                    nc.scalar.activation(
                        aggrB_part[:], ps_Sb[0][:], AF.Identity,
                        bias=sb[f"eb2p{l}"][:],
                    )
                    nc.vector.tensor_tensor(
                        aggrB_part[:], aggrB_part[:], ps_Sb[1][:], ALU.add
                    )
                    ci = dpool.tile([H, B], F32, tag=f"cc{l}_in")
                    co = dpool.tile([H, B], F32, tag=f"cc{l}_out")
                    dma4(ci, aggrB_part)
                    if variant == "nocc":
                        dma4(co, ci)
                    else:
                        nc.gpsimd.collective_compute(
                            "AllReduce", ALU.add, replica_groups=rg,
                            ins=[ci[:]], outs=[co[:]],
                        )
                    aggrB = wpool.tile([H, B], F32, tag="aggrB")
                    dma4(aggrB, co)
                    # bird node MLP (replicated)
                    h1b = wpool.tile([H, B], BF16, tag="h1b")
                    px2b = psA.tile([H, B], F32, tag="mm512")
                    xbT_new = wpool.tile([H, B], BF16, tag="xbT")
                    for k in range(4):
                        csl = slice(128 * k, 128 * (k + 1))
                        nc.tensor.matmul(
                            ph1b[:, csl], sb[f"n1a{l}"][:], aggrB[:, csl],
                            start=False, stop=True,
                        )
                        nc.scalar.activation(
                            h1b[:, csl], ph1b[:, csl], AF.Relu,
                            bias=sb[f"nb1_{l}"][:],
                        )
                        nc.tensor.matmul(
                            px2b[:, csl], sb[f"n2_16{l}"][:], h1b[:, csl]
                        )
                        nc.scalar.activation(
                            xbT_new[:, csl], px2b[:, csl], AF.Identity,
                            bias=sb[f"nb2_{l}"][:],
                        )
                    xbT = xbT_new
                    if not last:
                        xcT = xcT_new

                # ---- final: combined^T rows for my colors ----
                pssc = psA.tile([CPC, B], F32, tag="mm512")
                nc.tensor.matmul(pssc[:], sb["cpw16"][:], xbT[:])
                out_sb = wpool.tile([CPC, B], F32, tag="out_sb")
                nc.vector.scalar_tensor_tensor(
                    out=out_sb[:], in0=pssc[:], scalar=sb["cpb"][:],
                    in1=sb["probsT"][:], op0=ALU.add, op1=ALU.mult,
                )
                dma4(out_dram, out_sb)

            if variant == "nocc" and repeat > 1:
                with tc.For_i(0, repeat, 1) as _i:
                    emit_rep()
            else:
                for _rep in range(repeat):
                    emit_rep()

    nc.compile()
    return nc


_BUILT = {}


def _get_built(repeat: int = 1):
    if repeat not in _BUILT:
        _BUILT[repeat] = _build(repeat)
    return _BUILT[repeat]


def make_in_maps(probs, node_proj_w, node_proj_b, edge_w1, edge_b1, edge_w2,
                 edge_b2, node_w1, node_b1, node_w2, node_b2, color_proj_w,
                 color_proj_b):
    f = lambda x: np.ascontiguousarray(np.asarray(x, dtype=np.float32))
    g = lambda x: np.ascontiguousarray(np.asarray(x, dtype=np.float32).astype(_bf16))
    probs = f(probs)
    probsT = probs.T
    in_maps = []
    for c in range(NCORES):
        sl = slice(CPC * c, CPC * (c + 1))
        m = {
            "probsT": f(probsT[sl]),
            "probsTF": g(probsT),
            "npwF": g(node_proj_w),
            "xc0": f((np.asarray(node_proj_w)[sl] + np.asarray(node_proj_b)).T),
            "npb": f(node_proj_b).reshape(H, 1),
            "cpw16": g(np.asarray(color_proj_w)[:, sl]),
            "cpb": f(np.asarray(color_proj_b)[sl]).reshape(CPC, 1),
        }
        for l in range(L):
            m[f"e1t16{l}"] = g(edge_w1[l][:H])
            m[f"e1b{l}"] = f(edge_w1[l][H:])
            m[f"eb1_{l}"] = f(edge_b1[l]).reshape(H, 1)
            m[f"e2_{l}"] = f(edge_w2[l])
            m[f"eb2f{l}"] = f(512.0 * np.asarray(edge_b2[l])).reshape(H, 1)
            m[f"eb2p{l}"] = f(512.0 * np.asarray(edge_b2[l]) / 8.0).reshape(H, 1)
            m[f"n1x16{l}"] = g(node_w1[l][:H])
            m[f"n1x{l}"] = f(node_w1[l][:H])
            m[f"n1a{l}"] = f(node_w1[l][H:])
            m[f"nb1_{l}"] = f(node_b1[l]).reshape(H, 1)
            m[f"n2_16{l}"] = g(node_w2[l])
            m[f"n2_{l}"] = f(node_w2[l])
            m[f"nb2_{l}"] = f(node_b2[l]).reshape(H, 1)
        in_maps.append(m)
    return in_maps


def finish(probs, core_outs):
    combinedT = np.concatenate(core_outs, axis=0)  # (C, B)
    combined = combinedT.T  # (B, C)
    probs = np.asarray(probs, dtype=np.float32)
    idx = np.argsort(-probs, axis=1, kind="stable")
    cost = 1.0 - np.take_along_axis(combined, idx, axis=1)
    return cost.astype(np.float32)


def kernel(probs, node_proj_w, node_proj_b, edge_w1, edge_b1, edge_w2,
           edge_b2, node_w1, node_b1, node_w2, node_b2, color_proj_w,
           color_proj_b):
    nc = _get_built()
    in_maps = make_in_maps(
        probs, node_proj_w, node_proj_b, edge_w1, edge_b1, edge_w2, edge_b2,
        node_w1, node_b1, node_w2, node_b2, color_proj_w, color_proj_b,
    )
    res = bass_utils.run_bass_kernel_spmd(nc, in_maps, list(range(NCORES)))
    return finish(probs, [r["out"] for r in res.results])

